# revision 26
# baseline (speedup 1.0000x reference)
"""Cross-attention kernel for Trainium2, distributed over 8 NeuronCores.

Problem: B=4, Sk=4096, Sq=2048, d_model=1024, dims=64 (fp32 reference).

Sharding (hardcoded): core c -> (batch b = c//2, decoder half h = c%2).
Each core computes out[b, h*1024:(h+1)*1024, :] from enc[b] and its decoder
slice. No collectives.

Structure (v3):
  - bk dropped (constant per query row -> cancels in softmax). bv applied to
    the final normalized output. bq added during Q-projection evacuation on
    the Scalar engine (identity+bias), keeping DVE off the S critical path.
  - Weights packed host-side into one [1024, 256] tensor (Wv|Wk|Wq|Wq, Wq
    pre-scaled by dims^-0.5); consts packed into one [128, 65] f32.
  - KV^T projection: lhsT = [Wv|Wk], rhs = encT chunks -> psum [128, 512]
    (rows 0:64 V^T, 64:128 K^T). Both K^T and V^T are stored kb-PARITY
    split: even k-blocks on partitions 0:64, odd on 64:128, same column
    block. For S this makes consecutive k-blocks hit alternating PE row
    quadrants (concurrent matmuls); for V it makes ONE [128,128] PE
    transpose yield V-natural for TWO k-blocks.
  - Scores transposed S^T[k, q]; exp on ACT reads fp32 psum, writes bf16
    at tiles; AV accumulates out^T [65, 512] per q-half with the vnat ones
    column (memset once) summing the softmax denominator.
  - Pipeline per kb: S(kb) | kv-chunk production 1 chunk ahead | AV(kb-1).
    Program order IS the per-engine execution order.
  - Output: PE transpose of out^T blocks, DVE reciprocal, ACT
    identity-with-per-partition-scale normalizes + evacuates psum in one
    op, DVE adds bv, one DMA stores everything.
  - DMA: enc group 0 + dec interleaved as 0.5 MB pieces (2 KB contiguous
    lines), remaining enc as 1 MB halves, all on the Sync HWDGE queue;
    weights/consts on the gpsimd SWDGE queue.
"""

import numpy as np
import ml_dtypes

import concourse.bass as bass
import concourse.bacc as bacc
import concourse.tile as tile
from concourse import mybir
from concourse._compat import with_exitstack
from concourse.bass_utils import run_bass_kernel_spmd
from concourse.masks import make_identity

BF16 = mybir.dt.bfloat16
F32 = mybir.dt.float32

B, SK, SQ_FULL, D, DIMS = 4, 4096, 2048, 1024, 64
N_CORES = 8
SQ = SQ_FULL * B // N_CORES  # 1024 decoder rows per core
DC = D // 128  # d_model chunks of 128
KPAIRS = SK // 1024  # 4 enc 1024-col groups (esb tile pairs)
KBLKS = SK // 128  # 32 k blocks for attention
NCK = SK // 512  # 8 kv chunks
OBLKS = SQ // 128  # 8 output row blocks


@with_exitstack
def _body(ctx, tc, encT, decP, wkvq, consts, out):
    nc = tc.nc

    singles = ctx.enter_context(tc.tile_pool(name="singles", bufs=1))
    loads = ctx.enter_context(tc.tile_pool(name="loads", bufs=1))
    ps_pool = ctx.enter_context(tc.tile_pool(name="ps", bufs=2, space="PSUM"))
    po_pool = ctx.enter_context(tc.tile_pool(name="po", bufs=2, space="PSUM"))
    at_pool = ctx.enter_context(tc.tile_pool(name="at", bufs=6))
    outs = ctx.enter_context(tc.tile_pool(name="outs", bufs=3))

    # --- persistent activations ---
    # parity layout: even kb -> partitions 0:64 at col block kb//2, odd kb ->
    # partitions 64:128 same col block (for both K^T and V^T).
    kTd = singles.tile([128, KBLKS // 2 * 128], BF16)
    vTx = singles.tile([128, KBLKS // 2 * 128], BF16)
    # V natural blocks: vnat[p, kb, 0:64] = V[kb*128+p, :], col 64 = ones
    vnat = singles.tile([128, KBLKS, DIMS + 2], BF16)
    qTd = singles.tile([128, SQ], BF16)
    oT = singles.tile([DIMS + 1, SQ], F32)
    out_sb = singles.tile([128, OBLKS, DIMS], F32)

    # --- big activation streams first on the Sync HWDGE queue, in the order
    # compute consumes them. The host pre-packs enc/dec into piece-major
    # layouts so every 1 MB DMA reads a fully CONTIGUOUS DRAM range (8 KB
    # per partition) -> sequential HBM reads at full engine rate ---
    esbs = []
    for kp in range(KPAIRS):
        e0 = loads.tile([128, 4, 1024], BF16, tag=f"esb{kp}a", name=f"esb{kp}a")
        e1 = loads.tile([128, 4, 1024], BF16, tag=f"esb{kp}b", name=f"esb{kp}b")
        esbs.append((e0, e1))
    dsb = loads.tile([128, DC, SQ], BF16, tag="dload")

    def enc_piece(i):  # encT holds pieces [8, 128, 4, 1024] flattened
        return encT[i * 128:(i + 1) * 128, :, :]

    nc.sync.dma_start(out=esbs[0][0], in_=enc_piece(0))
    nc.sync.dma_start(out=esbs[0][1], in_=enc_piece(1))
    nc.sync.dma_start(out=dsb[:, 0:4, :], in_=decP[0:128, :, :])
    nc.sync.dma_start(out=dsb[:, 4:8, :], in_=decP[128:256, :, :])
    for kp in range(1, KPAIRS):
        nc.sync.dma_start(out=esbs[kp][0], in_=enc_piece(2 * kp))
        nc.sync.dma_start(out=esbs[kp][1], in_=enc_piece(2 * kp + 1))

    # --- identities first (gpsimd) so PE warm-up can start early, then
    # constants on the gpsimd SWDGE queue ---
    ident = singles.tile([128, 128], F32)
    make_identity(nc, ident)
    ident_bf = singles.tile([128, 128], BF16)
    make_identity(nc, ident_bf)
    wkvq_sb = singles.tile([128, DC, 256], BF16)
    nc.gpsimd.dma_start(out=wkvq_sb, in_=wkvq.rearrange("(c p) m -> p c m", p=128))
    consts_sb = singles.tile([128, 1 + DIMS], F32)
    nc.gpsimd.dma_start(out=consts_sb, in_=consts)
    nc.gpsimd.memset(vnat[:, :, DIMS:DIMS + 1], 1.0)

    # --- K/V projection for one 512-col chunk (4 k blocks): project + evac
    # to parity layouts; V transposes run 2 kb later (v_trans) so their
    # weight loads never wait on the fresh DVE casts ---
    def kv_proj(ck):
        pskv = ps_pool.tile([128, 512], F32, tag="aux", name=f"pskv{ck % 2}")
        for d in range(DC):
            esb = esbs[ck // 2][d // 4]
            nc.tensor.matmul(
                pskv, lhsT=wkvq_sb[:, d, 0:128],
                rhs=esb[:, d % 4, (ck % 2) * 512:(ck % 2 + 1) * 512],
                start=(d == 0), stop=(d == DC - 1),
            )
        # parity evac: kb 4ck,4ck+2 -> rows 0:64; 4ck+1,4ck+3 -> rows 64:128
        csl = slice(ck * 256, (ck + 1) * 256)
        vv = pskv[0:DIMS, :].rearrange("p (a b n) -> p a b n", a=2, b=2)
        kk = pskv[DIMS:128, :].rearrange("p (a b n) -> p a b n", a=2, b=2)
        nc.vector.tensor_copy(
            vTx[0:DIMS, csl].rearrange("p (a n) -> p a n", a=2), vv[:, :, 0, :]
        )
        nc.vector.tensor_copy(
            vTx[DIMS:128, csl].rearrange("p (a n) -> p a n", a=2), vv[:, :, 1, :]
        )
        nc.vector.tensor_copy(
            kTd[0:DIMS, csl].rearrange("p (a n) -> p a n", a=2), kk[:, :, 0, :]
        )
        nc.vector.tensor_copy(
            kTd[DIMS:128, csl].rearrange("p (a n) -> p a n", a=2), kk[:, :, 1, :]
        )

    def v_trans(ck):
        for pb in (2 * ck, 2 * ck + 1):  # one transpose per kb pair
            ptv = ps_pool.tile([128, 128], BF16, tag="aux", name=f"ptv{pb % 2}")
            nc.tensor.transpose(
                ptv, vTx[:, pb * 128:(pb + 1) * 128], ident_bf
            )
            nc.vector.tensor_copy(
                vnat[:, 2 * pb:2 * pb + 2, 0:DIMS],
                ptv.rearrange("p (a n) -> p a n", a=2),
            )

    # --- attention for a kb pair: 4 S matmuls interleaved even/odd so
    # consecutive matmuls hit alternating PE row quadrants (concurrent),
    # 2 exps, then 4 AV matmuls (issued a pair later) ---
    po0 = po_pool.tile([DIMS + 1, 512], F32, tag="po")
    po1 = po_pool.tile([DIMS + 1, 512], F32, tag="po")
    pos = [po0, po1]

    at_tiles = {}

    def s_pair(kp):
        e, o = 2 * kp, 2 * kp + 1
        kt_e = kTd[0:DIMS, kp * 128:(kp + 1) * 128]
        kt_o = kTd[DIMS:128, kp * 128:(kp + 1) * 128]
        pss_e = ps_pool.tile([128, 2, 512], F32, tag="ps", name="pss0")
        pss_o = ps_pool.tile([128, 2, 512], F32, tag="ps", name="pss1")
        for j in range(2):
            nc.tensor.matmul(
                pss_e[:, j, :], lhsT=kt_e,
                rhs=qTd[0:DIMS, j * 512:(j + 1) * 512],
                start=True, stop=True,
            )
            nc.tensor.matmul(
                pss_o[:, j, :], lhsT=kt_o,
                rhs=qTd[DIMS:128, j * 512:(j + 1) * 512],
                start=True, stop=True,
            )
        for kb, pss in ((e, pss_e), (o, pss_o)):
            at = at_pool.tile([128, 2, 512], BF16, tag="at", name=f"at{kb % 6}")
            at_tiles[kb] = at
            nc.scalar.activation(at, pss, mybir.ActivationFunctionType.Exp)

    def av(kb, js=(0, 1), pop=True):
        at = at_tiles.pop(kb) if pop else at_tiles[kb]
        st = kb == 0
        sp = kb == KBLKS - 1
        for j in js:
            nc.tensor.matmul(
                pos[j], lhsT=vnat[:, kb, 0:DIMS + 1], rhs=at[:, j, :],
                start=st, stop=sp,
            )

    # --- warm-up: independent matmuls on the identity keep the PE busy
    # during the initial DMA wait so HAM un-throttles before real work ---
    warm = ps_pool.tile([128, 128], F32, tag="aux", name="warm")
    for _ in range(50):
        nc.tensor.matmul(warm, lhsT=ident_bf, rhs=ident_bf, start=True, stop=True)

    # --- prologue: kv chunks 0/1 fill the decoder DMA wait, then the Q
    # projection streams with dec arrival (bias add on ACT) ---
    psq = ps_pool.tile([128, 2, 512], F32, tag="ps", name="pss0")
    kv_proj(0)
    kv_proj(1)
    v_trans(0)
    for j in range(2):
        for d in range(DC):
            nc.tensor.matmul(
                psq[:, j, :], lhsT=wkvq_sb[:, d, 128:256],
                rhs=dsb[:, d, j * 512:(j + 1) * 512],
                start=(d == 0), stop=(d == DC - 1),
            )
        nc.scalar.activation(
            qTd[:, j * 512:(j + 1) * 512], psq[:, j, :],
            mybir.ActivationFunctionType.Identity, bias=consts_sb[:, 0:1],
        )
        if j == 0:
            v_trans(1)

    # --- pipeline over kb pairs: S+exp(pair kp) | kv chunk production two
    # chunks ahead (projection at even kp, transposes at odd kp) | AV(kp-1) ---
    for kp in range(KBLKS // 2):
        s_pair(kp)
        if kp % 2 == 0 and kp // 2 + 2 < NCK:
            kv_proj(kp // 2 + 2)
        if kp % 2 == 1 and (kp + 3) // 2 < NCK:
            v_trans((kp + 3) // 2)
        if kp > 0:
            av(2 * kp - 2)
            av(2 * kp - 1)
    # j-split end-game: finish pos[0] first so its output half overlaps the
    # remaining j=1 AV matmuls
    av(KBLKS - 2, js=(0,), pop=False)
    av(KBLKS - 1, js=(0,), pop=False)
    nc.vector.tensor_copy(oT[:, 0:512], pos[0])
    av(KBLKS - 2, js=(1,), pop=False)
    av(KBLKS - 1, js=(1,))
    nc.vector.tensor_copy(oT[:, 512:1024], pos[1])

    # --- output: transpose oT blocks, normalize via ACT scale, add bv ---
    for ob in range(OBLKS):
        pt = ps_pool.tile([128, DIMS + 1], F32, tag="aux", name=f"pt{ob % 2}")
        nc.tensor.transpose(
            pt, oT[:, ob * 128:(ob + 1) * 128], ident[0:DIMS + 1, 0:DIMS + 1]
        )
        rcp = outs.tile([128, 1], F32, tag="rcp", name=f"rcp{ob % 2}")
        nc.vector.reciprocal(rcp, pt[:, DIMS:DIMS + 1])
        nc.scalar.activation(
            out_sb[:, ob, :], pt[:, 0:DIMS],
            mybir.ActivationFunctionType.Identity, scale=rcp,
        )
        nc.vector.tensor_add(
            out_sb[:, ob, :], out_sb[:, ob, :], consts_sb[:, 1:1 + DIMS]
        )
    nc.sync.dma_start(
        out=out.rearrange("(b p) d -> p b d", p=128), in_=out_sb
    )


_NC_CACHE = None


def _build():
    global _NC_CACHE
    if _NC_CACHE is not None:
        return _NC_CACHE
    nc = bacc.Bacc(
        "TRN2", target_bir_lowering=False, debug=False,
        enable_asserts=True, num_devices=N_CORES,
    )
    encT = nc.dram_tensor("encT", [8 * 128, 4, 1024], BF16, kind="ExternalInput").ap()
    decP = nc.dram_tensor("decP", [2 * 128, 4, 1024], BF16, kind="ExternalInput").ap()
    wkvq = nc.dram_tensor("wkvq", [D, 256], BF16, kind="ExternalInput").ap()
    consts = nc.dram_tensor("consts", [128, 1 + DIMS], F32, kind="ExternalInput").ap()
    out = nc.dram_tensor("out", [SQ, DIMS], F32, kind="ExternalOutput").ap()
    with tile.TileContext(nc) as tc:
        _body(tc, encT, decP, wkvq, consts, out)
    nc.compile()
    _NC_CACHE = nc
    return nc


def make_in_maps(**inputs):
    bf16 = ml_dtypes.bfloat16
    enc = np.asarray(inputs["encoder_output"])
    dec = np.asarray(inputs["decoder"])
    scale = DIMS ** -0.5
    wq1 = np.asarray(inputs["Wq"]) * scale
    wkvq = np.concatenate(
        [np.asarray(inputs["Wv"]), np.asarray(inputs["Wk"]), wq1, wq1], axis=1
    ).astype(bf16)
    bq1 = (np.asarray(inputs["bq"]) * scale).astype(np.float32).reshape(DIMS, 1)
    consts = np.zeros((128, 1 + DIMS), np.float32)
    consts[0:DIMS, 0:1] = bq1
    consts[DIMS:128, 0:1] = bq1
    consts[:, 1:1 + DIMS] = np.asarray(inputs["bv"]).astype(np.float32).reshape(1, DIMS)
    in_maps = []
    for c in range(N_CORES):
        b, h = divmod(c, 2)
        # piece-major contiguous layouts: enc piece 2*kp+hf fills
        # esb[kp][hf][p, cc, n] = encT[(4*hf+cc)*128+p, kp*1024+n]
        enc_cpn = np.ascontiguousarray(enc[b].T.astype(bf16)).reshape(8, 128, SK)
        encP = np.empty((8, 128, 4, 1024), bf16)
        for kp in range(4):
            for hf in range(2):
                encP[2 * kp + hf] = enc_cpn[
                    4 * hf:4 * hf + 4, :, kp * 1024:(kp + 1) * 1024
                ].transpose(1, 0, 2)
        dec_cpn = np.ascontiguousarray(
            dec[b, h * SQ:(h + 1) * SQ, :].T.astype(bf16)
        ).reshape(8, 128, SQ)
        decPp = np.empty((2, 128, 4, SQ), bf16)
        for hf in range(2):
            decPp[hf] = dec_cpn[4 * hf:4 * hf + 4].transpose(1, 0, 2)
        in_maps.append({
            "encT": encP.reshape(8 * 128, 4, 1024),
            "decP": decPp.reshape(2 * 128, 4, SQ),
            "wkvq": wkvq, "consts": consts,
        })
    return in_maps


def assemble(results):
    out = np.zeros((B, SQ_FULL, DIMS), np.float32)
    for c in range(N_CORES):
        b, h = divmod(c, 2)
        out[b, h * SQ:(h + 1) * SQ] = results[c]["out"]
    return out


def kernel(**inputs) -> np.ndarray:
    nc = _build()
    in_maps = make_in_maps(**inputs)
    res = run_bass_kernel_spmd(nc, in_maps, core_ids=list(range(N_CORES)))
    return assemble(res.results)


# revision 34
# speedup vs baseline: 1.0313x; 1.0313x over previous
"""Cross-attention kernel for Trainium2, distributed over 8 NeuronCores.

Problem: B=4, Sk=4096, Sq=2048, d_model=1024, dims=64 (fp32 reference).

Sharding (hardcoded): core c -> (batch b = c//2, decoder half h = c%2).
Each core computes out[b, h*1024:(h+1)*1024, :] from enc[b] and its decoder
slice. No collectives.

Structure (v3):
  - bk dropped (constant per query row -> cancels in softmax). bv applied to
    the final normalized output. bq added during Q-projection evacuation on
    the Scalar engine (identity+bias), keeping DVE off the S critical path.
  - Weights packed host-side into one [1024, 256] tensor (Wv|Wk|Wq|Wq, Wq
    pre-scaled by dims^-0.5); consts packed into one [128, 65] f32.
  - KV^T projection: lhsT = [Wv|Wk], rhs = encT chunks -> psum [128, 512]
    (rows 0:64 V^T, 64:128 K^T). Both K^T and V^T are stored kb-PARITY
    split: even k-blocks on partitions 0:64, odd on 64:128, same column
    block. For S this makes consecutive k-blocks hit alternating PE row
    quadrants (concurrent matmuls); for V it makes ONE [128,128] PE
    transpose yield V-natural for TWO k-blocks.
  - Scores transposed S^T[k, q]; exp on ACT reads fp32 psum, writes bf16
    at tiles; AV accumulates out^T [65, 512] per q-half with the vnat ones
    column (memset once) summing the softmax denominator.
  - Pipeline per kb: S(kb) | kv-chunk production 1 chunk ahead | AV(kb-1).
    Program order IS the per-engine execution order.
  - Output: PE transpose of out^T blocks, DVE reciprocal, ACT
    identity-with-per-partition-scale normalizes + evacuates psum in one
    op, DVE adds bv, one DMA stores everything.
  - DMA: enc group 0 + dec interleaved as 0.5 MB pieces (2 KB contiguous
    lines), remaining enc as 1 MB halves, all on the Sync HWDGE queue;
    weights/consts on the gpsimd SWDGE queue.
"""

import numpy as np
import ml_dtypes

import concourse.bass as bass
import concourse.bacc as bacc
import concourse.tile as tile
from concourse import mybir
from concourse._compat import with_exitstack
from concourse.bass_utils import run_bass_kernel_spmd
from concourse.masks import make_identity

BF16 = mybir.dt.bfloat16
F32 = mybir.dt.float32

B, SK, SQ_FULL, D, DIMS = 4, 4096, 2048, 1024, 64
N_CORES = 8
SQ = SQ_FULL * B // N_CORES  # 1024 decoder rows per core
DC = D // 128  # d_model chunks of 128
KPAIRS = SK // 1024  # 4 enc 1024-col groups (esb tile pairs)
KBLKS = SK // 128  # 32 k blocks for attention
NCK = SK // 512  # 8 kv chunks
OBLKS = SQ // 128  # 8 output row blocks


@with_exitstack
def _body(ctx, tc, encT, decP, wkvq, consts, out):
    nc = tc.nc

    singles = ctx.enter_context(tc.tile_pool(name="singles", bufs=1))
    loads = ctx.enter_context(tc.tile_pool(name="loads", bufs=1))
    ps_pool = ctx.enter_context(tc.tile_pool(name="ps", bufs=2, space="PSUM"))
    po_pool = ctx.enter_context(tc.tile_pool(name="po", bufs=2, space="PSUM"))
    at_pool = ctx.enter_context(tc.tile_pool(name="at", bufs=6))
    outs = ctx.enter_context(tc.tile_pool(name="outs", bufs=3))

    # --- persistent activations ---
    # parity layout: even kb -> partitions 0:64 at col block kb//2, odd kb ->
    # partitions 64:128 same col block (for both K^T and V^T).
    kTd = singles.tile([128, KBLKS // 2 * 128], BF16)
    vTx = singles.tile([128, KBLKS // 2 * 128], BF16)
    # V natural blocks: vnat[p, kb, 0:64] = V[kb*128+p, :], col 64 = ones
    vnat = singles.tile([128, KBLKS, DIMS + 2], BF16)
    qTd = singles.tile([128, SQ], BF16)
    oT = singles.tile([DIMS + 1, SQ], F32)
    out_sb = singles.tile([128, OBLKS, DIMS], F32)

    # --- big activation streams first on the Sync HWDGE queue, in the order
    # compute consumes them. The host pre-packs enc/dec into piece-major
    # layouts so every 1 MB DMA reads a fully CONTIGUOUS DRAM range (8 KB
    # per partition) -> sequential HBM reads at full engine rate ---
    esbs = []
    for kp in range(KPAIRS):
        e0 = loads.tile([128, 4, 1024], BF16, tag=f"esb{kp}a", name=f"esb{kp}a")
        e1 = loads.tile([128, 4, 1024], BF16, tag=f"esb{kp}b", name=f"esb{kp}b")
        esbs.append((e0, e1))
    dsb = loads.tile([128, DC, SQ], BF16, tag="dload")

    enc_r = encT.rearrange("(c p) n -> p c n", p=128)
    dec_r = decP.rearrange("(c p) n -> p c n", p=128)
    # interleave group-0 and decoder pieces so the prologue's kv and Q
    # matmuls can both make progress as data arrives
    for half in range(2):
        for h in (2 * half, 2 * half + 1):
            nc.sync.dma_start(
                out=esbs[0][h // 2][:, (h % 2) * 2:(h % 2) * 2 + 2, :],
                in_=enc_r[:, h * 2:h * 2 + 2, 0:1024],
            )
        for h in (2 * half, 2 * half + 1):
            nc.sync.dma_start(
                out=dsb[:, h * 2:h * 2 + 2, :], in_=dec_r[:, h * 2:h * 2 + 2, :]
            )
    for kp in range(1, KPAIRS):
        sl = slice(kp * 1024, (kp + 1) * 1024)
        nc.sync.dma_start(out=esbs[kp][0], in_=enc_r[:, 0:4, sl])
        nc.sync.dma_start(out=esbs[kp][1], in_=enc_r[:, 4:8, sl])

    # --- identities first (gpsimd) so PE warm-up can start early, then
    # constants on the gpsimd SWDGE queue ---
    ident = singles.tile([128, 128], F32)
    make_identity(nc, ident)
    ident_bf = singles.tile([128, 128], BF16)
    make_identity(nc, ident_bf)
    wkvq_sb = singles.tile([128, DC, 256], BF16)
    nc.gpsimd.dma_start(out=wkvq_sb, in_=wkvq.rearrange("(c p) m -> p c m", p=128))
    consts_sb = singles.tile([128, 1 + DIMS], F32)
    nc.gpsimd.dma_start(out=consts_sb, in_=consts)
    nc.gpsimd.memset(vnat[:, :, DIMS:DIMS + 1], 1.0)

    # --- K/V projection for one 512-col chunk (4 k blocks): project + evac
    # to parity layouts; V transposes run 2 kb later (v_trans) so their
    # weight loads never wait on the fresh DVE casts ---
    def kv_proj(ck):
        pskv = ps_pool.tile([128, 512], F32, tag="aux", name=f"pskv{ck % 2}")
        for d in range(DC):
            esb = esbs[ck // 2][d // 4]
            nc.tensor.matmul(
                pskv, lhsT=wkvq_sb[:, d, 0:128],
                rhs=esb[:, d % 4, (ck % 2) * 512:(ck % 2 + 1) * 512],
                start=(d == 0), stop=(d == DC - 1),
            )
        # parity evac: kb 4ck,4ck+2 -> rows 0:64; 4ck+1,4ck+3 -> rows 64:128
        csl = slice(ck * 256, (ck + 1) * 256)
        vv = pskv[0:DIMS, :].rearrange("p (a b n) -> p a b n", a=2, b=2)
        kk = pskv[DIMS:128, :].rearrange("p (a b n) -> p a b n", a=2, b=2)
        nc.vector.tensor_copy(
            vTx[0:DIMS, csl].rearrange("p (a n) -> p a n", a=2), vv[:, :, 0, :]
        )
        nc.vector.tensor_copy(
            vTx[DIMS:128, csl].rearrange("p (a n) -> p a n", a=2), vv[:, :, 1, :]
        )
        nc.vector.tensor_copy(
            kTd[0:DIMS, csl].rearrange("p (a n) -> p a n", a=2), kk[:, :, 0, :]
        )
        nc.vector.tensor_copy(
            kTd[DIMS:128, csl].rearrange("p (a n) -> p a n", a=2), kk[:, :, 1, :]
        )

    def v_trans(ck):
        for pb in (2 * ck, 2 * ck + 1):  # one transpose per kb pair
            ptv = ps_pool.tile([128, 128], BF16, tag="aux", name=f"ptv{pb % 2}")
            nc.tensor.transpose(
                ptv, vTx[:, pb * 128:(pb + 1) * 128], ident_bf
            )
            nc.vector.tensor_copy(
                vnat[:, 2 * pb:2 * pb + 2, 0:DIMS],
                ptv.rearrange("p (a n) -> p a n", a=2),
            )

    # --- attention for a kb pair: 4 S matmuls interleaved even/odd so
    # consecutive matmuls hit alternating PE row quadrants (concurrent),
    # 2 exps, then 4 AV matmuls (issued a pair later) ---
    po0 = po_pool.tile([DIMS + 1, 512], F32, tag="po")
    po1 = po_pool.tile([DIMS + 1, 512], F32, tag="po")
    pos = [po0, po1]

    at_tiles = {}

    def s_pair(kp):
        e, o = 2 * kp, 2 * kp + 1
        kt_e = kTd[0:DIMS, kp * 128:(kp + 1) * 128]
        kt_o = kTd[DIMS:128, kp * 128:(kp + 1) * 128]
        pss_e = ps_pool.tile([128, 2, 512], F32, tag="ps", name="pss0")
        pss_o = ps_pool.tile([128, 2, 512], F32, tag="ps", name="pss1")
        for j in range(2):
            nc.tensor.matmul(
                pss_e[:, j, :], lhsT=kt_e,
                rhs=qTd[0:DIMS, j * 512:(j + 1) * 512],
                start=True, stop=True,
            )
            nc.tensor.matmul(
                pss_o[:, j, :], lhsT=kt_o,
                rhs=qTd[DIMS:128, j * 512:(j + 1) * 512],
                start=True, stop=True,
            )
        for kb, pss in ((e, pss_e), (o, pss_o)):
            at = at_pool.tile([128, 2, 512], BF16, tag="at", name=f"at{kb % 6}")
            at_tiles[kb] = at
            nc.scalar.activation(at, pss, mybir.ActivationFunctionType.Exp)

    def av(kb, js=(0, 1), pop=True):
        at = at_tiles.pop(kb) if pop else at_tiles[kb]
        st = kb == 0
        sp = kb == KBLKS - 1
        for j in js:
            nc.tensor.matmul(
                pos[j], lhsT=vnat[:, kb, 0:DIMS + 1], rhs=at[:, j, :],
                start=st, stop=sp,
            )

    # --- warm-up: independent matmuls on the identity keep the PE busy
    # during the initial DMA wait so HAM un-throttles before real work ---
    warm = ps_pool.tile([128, 128], F32, tag="aux", name="warm")
    for _ in range(50):
        nc.tensor.matmul(warm, lhsT=ident_bf, rhs=ident_bf, start=True, stop=True)

    # --- prologue: kv chunks 0/1 fill the decoder DMA wait, then the Q
    # projection streams with dec arrival (bias add on ACT) ---
    psq = ps_pool.tile([128, 2, 512], F32, tag="ps", name="pss0")
    kv_proj(0)
    kv_proj(1)
    v_trans(0)
    for j in range(2):
        for d in range(DC):
            nc.tensor.matmul(
                psq[:, j, :], lhsT=wkvq_sb[:, d, 128:256],
                rhs=dsb[:, d, j * 512:(j + 1) * 512],
                start=(d == 0), stop=(d == DC - 1),
            )
        nc.scalar.activation(
            qTd[:, j * 512:(j + 1) * 512], psq[:, j, :],
            mybir.ActivationFunctionType.Identity, bias=consts_sb[:, 0:1],
        )
        if j == 0:
            v_trans(1)

    # --- pipeline over kb pairs: S+exp(pair kp) | kv chunk production two
    # chunks ahead (projection at even kp, transposes at odd kp) | AV(kp-1) ---
    for kp in range(KBLKS // 2):
        s_pair(kp)
        if kp % 2 == 0 and kp // 2 + 2 < NCK:
            kv_proj(kp // 2 + 2)
        if kp % 2 == 1 and (kp + 3) // 2 < NCK:
            v_trans((kp + 3) // 2)
        if kp > 0:
            av(2 * kp - 2)
            av(2 * kp - 1)
    # j-split end-game: finish pos[0] first so its output half overlaps the
    # remaining j=1 AV matmuls
    av(KBLKS - 2, js=(0,), pop=False)
    av(KBLKS - 1, js=(0,), pop=False)
    nc.vector.tensor_copy(oT[:, 0:512], pos[0])
    av(KBLKS - 2, js=(1,), pop=False)
    av(KBLKS - 1, js=(1,))
    nc.vector.tensor_copy(oT[:, 512:1024], pos[1])

    # --- output: transpose oT blocks, normalize via ACT scale, add bv ---
    for ob in range(OBLKS):
        pt = ps_pool.tile([128, DIMS + 1], F32, tag="aux", name=f"pt{ob % 2}")
        nc.tensor.transpose(
            pt, oT[:, ob * 128:(ob + 1) * 128], ident[0:DIMS + 1, 0:DIMS + 1]
        )
        rcp = outs.tile([128, 1], F32, tag="rcp", name=f"rcp{ob % 2}")
        nc.vector.reciprocal(rcp, pt[:, DIMS:DIMS + 1])
        nc.scalar.activation(
            out_sb[:, ob, :], pt[:, 0:DIMS],
            mybir.ActivationFunctionType.Identity, scale=rcp,
        )
        nc.vector.tensor_add(
            out_sb[:, ob, :], out_sb[:, ob, :], consts_sb[:, 1:1 + DIMS]
        )
    nc.sync.dma_start(
        out=out.rearrange("(b p) d -> p b d", p=128), in_=out_sb
    )


_NC_CACHE = None


def _build():
    global _NC_CACHE
    if _NC_CACHE is not None:
        return _NC_CACHE
    nc = bacc.Bacc(
        "TRN2", target_bir_lowering=False, debug=False,
        enable_asserts=True, num_devices=N_CORES,
    )
    encT = nc.dram_tensor("encT", [D, SK], BF16, kind="ExternalInput").ap()
    decP = nc.dram_tensor("decP", [D, SQ], BF16, kind="ExternalInput").ap()
    wkvq = nc.dram_tensor("wkvq", [D, 256], BF16, kind="ExternalInput").ap()
    consts = nc.dram_tensor("consts", [128, 1 + DIMS], F32, kind="ExternalInput").ap()
    out = nc.dram_tensor("out", [SQ, DIMS], F32, kind="ExternalOutput").ap()
    with tile.TileContext(nc) as tc:
        _body(tc, encT, decP, wkvq, consts, out)
    nc.compile()
    _NC_CACHE = nc
    return nc


def make_in_maps(**inputs):
    bf16 = ml_dtypes.bfloat16
    enc = np.asarray(inputs["encoder_output"])
    dec = np.asarray(inputs["decoder"])
    scale = DIMS ** -0.5
    wq1 = np.asarray(inputs["Wq"]) * scale
    wkvq = np.concatenate(
        [np.asarray(inputs["Wv"]), np.asarray(inputs["Wk"]), wq1, wq1], axis=1
    ).astype(bf16)
    bq1 = (np.asarray(inputs["bq"]) * scale).astype(np.float32).reshape(DIMS, 1)
    consts = np.zeros((128, 1 + DIMS), np.float32)
    consts[0:DIMS, 0:1] = bq1
    consts[DIMS:128, 0:1] = bq1
    consts[:, 1:1 + DIMS] = np.asarray(inputs["bv"]).astype(np.float32).reshape(1, DIMS)
    in_maps = []
    for c in range(N_CORES):
        b, h = divmod(c, 2)
        in_maps.append({
            "encT": enc[b].T.astype(bf16),
            "decP": dec[b, h * SQ:(h + 1) * SQ, :].T.astype(bf16),
            "wkvq": wkvq, "consts": consts,
        })
    return in_maps


def assemble(results):
    out = np.zeros((B, SQ_FULL, DIMS), np.float32)
    for c in range(N_CORES):
        b, h = divmod(c, 2)
        out[b, h * SQ:(h + 1) * SQ] = results[c]["out"]
    return out


def kernel(**inputs) -> np.ndarray:
    nc = _build()
    in_maps = make_in_maps(**inputs)
    res = run_bass_kernel_spmd(nc, in_maps, core_ids=list(range(N_CORES)))
    return assemble(res.results)
